# revision 40
# baseline (speedup 1.0000x reference)
"""Dense-MoE (all experts, softmax-gated) Trainium2 kernel — fp16 I/O, v4.

Math reformulation (per token t):
  s1    = xT @ [Wd_cat | Wg]            # one K=768 matmul -> [64 h1 | 8 logits]
  e     = exp(s1[64:72] + bg)           # unnormalized gate
  h1b   = s1[:64] + bd
  g64   = expand(e)                     # K=8 matmul vs 0/1 matrix
  s3in  = [h1b * g64 ; e]               # gate folded BEFORE Wm (block-diag)
  o     = s3in @ [[0, Wm@Wu], [1, bu + bm@Wu]]  # stage-2 folded into stage-3
  out   = o[2:] / o[0]                  # softmax normalization at the end

The middle matmul (block-diag Wm) and its bias commute with the per-expert
gate scale, so Wm@Wu and bm@Wu are precomputed on the host and stage 2
disappears from the device entirely.

I/O precision: x is pre-transposed and cast to fp16 on the host (no PE
transposes), weights fp16, output written fp16 and upcast on the host.
PSUM accumulation stays fp32; rel err ~4e-4.

LDWEIGHTS fillers keep the PE HAM duty-cycle monitor from dropping the
clock to 1.2GHz (it throttles when PE activity per 3.4us window is low).

Sharding: data-parallel over tokens, 8 cores, weights replicated.
"""

import numpy as np

B, S, D, E, R = 8, 4096, 768, 8, 8
NCORES = 8
T_CORE = B * S // NCORES          # 4096 tokens per core
TILE_T = 512                      # tokens per compute tile
N_TILES = T_CORE // TILE_T        # 8
EW = E * R                        # 64
KW = EW + E                       # 72
KC = D // 128                     # 6 contraction chunks for stage 1
JC = TILE_T // 128                # 4 token chunks of 128 per tile
W3N = 2 + D                       # 770 stage-3 columns (2 x Z, then 768 out)
NW = KC * KW + EW + W3N + 2       # 1268 packed fp16 weight cols (last 2: bd, bg)

FILL_MID = 1                      # pacing filler matmuls (HAM warmers)
FILL_CHUNK = (1, 2)               # chunks after which to insert one filler
FILL_END = 2
N_WARM = 9

_CACHE = {}


def _build_and_compile():
    """Build the Bass/Tile program once. Returns compiled nc."""
    from contextlib import ExitStack

    import concourse.bass as bass
    import concourse.tile as tile
    from concourse import bacc, mybir

    f32 = mybir.dt.float32
    f16 = mybir.dt.float16
    AF = mybir.ActivationFunctionType
    ALU = mybir.AluOpType

    nc = bacc.Bacc("TRN2", target_bir_lowering=False, debug=False, num_devices=NCORES)

    xt_d = nc.dram_tensor(
        "xt", [128, N_TILES * KC * TILE_T], f16, kind="ExternalInput"
    ).ap()
    wp_d = nc.dram_tensor("wpack", [128, NW], f16, kind="ExternalInput").ap()
    out_d = nc.dram_tensor("out", [T_CORE, D], f16, kind="ExternalOutput").ap()

    xt_p = xt_d.rearrange("p (i k) -> p i k", i=N_TILES)         # [128,NT,KC*T]
    out_v = out_d.rearrange("(i j p) d -> i p j d", j=JC, p=128)

    with tile.TileContext(nc) as tc, ExitStack() as ctx:
        const = ctx.enter_context(tc.tile_pool(name="const", bufs=1))
        xin = ctx.enter_context(tc.tile_pool(name="xin", bufs=4))
        mid_p = ctx.enter_context(tc.tile_pool(name="mid", bufs=2))
        outp = ctx.enter_context(tc.tile_pool(name="outp", bufs=3))
        small = ctx.enter_context(tc.tile_pool(name="small", bufs=4))
        # PSUM budget (8 banks): s1 2 + g64 1 + s3ab 2x2 + fill 1 = 8
        s1p = ctx.enter_context(tc.tile_pool(name="s1p", bufs=2, space="PSUM"))
        g64p = ctx.enter_context(tc.tile_pool(name="g64p", bufs=1, space="PSUM"))
        s3ap = ctx.enter_context(tc.tile_pool(name="s3ap", bufs=2, space="PSUM"))
        fillp = ctx.enter_context(tc.tile_pool(name="fillp", bufs=1, space="PSUM"))

        # x(0) first (it is the long pole), then weights — all on the sync
        # ring (DMA completion shares semaphores, keep SBUF writes ordered).
        x_sb0 = xin.tile([128, KC * TILE_T], f16, name="x_sb0", tag="x")
        nc.sync.dma_start(x_sb0[:], xt_p[:, 0, :])
        wp = const.tile([128, NW], f16, name="wp")
        nc.sync.dma_start(wp[:], wp_d)

        c0 = 0
        w1_sb = wp[:, c0:c0 + KC * KW]; c0 += KC * KW
        e8_sb = wp[EW:KW, c0:c0 + EW]; c0 += EW
        w3_sb = wp[0:KW, c0:c0 + W3N]; c0 += W3N
        bd_sb = wp[0:EW, c0:c0 + 1]; c0 += 1
        bg_sb = wp[0:E, c0:c0 + 1]; c0 += 1

        # HAM pre-warm: fp32 matmuls (zero data, results unused, no DMA
        # dependency) so the PE clock is at 2.4GHz when tile 0 arrives.
        warm_src = const.tile([128, 128], f32, name="warm_src")
        nc.gpsimd.memset(warm_src[:], 0.0)
        warm_ps = s1p.tile([128, TILE_T], f32, name="warm_ps", tag="s1")
        for _k in range(N_WARM):
            nc.tensor.matmul(
                warm_ps[:, 0:128], warm_src[:], warm_src[:], start=True, stop=True
            )
        fill_ps = fillp.tile([128, TILE_T], f32, name="fill_ps", tag="fill")

        def filler(xt_sb, n):
            # fp16 matmul on live tile data: keeps the PE array streaming so
            # the HAM duty monitor never drops the clock; result never read.
            for _f in range(n):
                nc.tensor.matmul(
                    fill_ps[:], xt_sb[:, 0:128], xt_sb[:, 0:TILE_T],
                    start=True, stop=True,
                )

        x_sbs, s1s, s3ins, h1bs = {}, {}, {}, {}

        def load(i):
            if i == 0:
                x_sbs[0] = x_sb0
                return
            x_sb = xin.tile([128, KC * TILE_T], f16, name="x_sb", tag="x")
            nc.sync.dma_start(x_sb[:], xt_p[:, i, :])
            x_sbs[i] = x_sb

        def mid1(i):
            """stage 1 matmuls + exp + biased h1 (both scalar-engine)."""
            xt_sb = x_sbs[i]
            s1 = s1p.tile([KW, TILE_T], f32, name="s1", tag="s1")
            for c in range(KC):
                nc.tensor.matmul(
                    s1[:],
                    w1_sb[:, c * KW:(c + 1) * KW],
                    xt_sb[:, c * TILE_T:(c + 1) * TILE_T],
                    start=(c == 0),
                    stop=(c == KC - 1),
                )
            filler(xt_sb, FILL_MID)
            s3in = mid_p.tile([KW, TILE_T], f16, name="s3in", tag="s3in")
            nc.scalar.activation(s3in[EW:KW, :], s1[EW:KW, :], AF.Exp, bias=bg_sb[:])
            h1b = mid_p.tile([EW, TILE_T], f16, name="h1b", tag="h1b")
            nc.scalar.activation(h1b[:], s1[0:EW, :], AF.Identity, bias=bd_sb[:])
            s1s[i], s3ins[i], h1bs[i] = s1, s3in, h1b

        def mid2(i):
            """gate expand matmul + gated h1 -> s3in rows 0:64 (one DVE op)."""
            s3in, h1b = s3ins[i], h1bs.pop(i)
            s1s.pop(i)
            g64_ps = g64p.tile([EW, TILE_T], f32, name="g64_ps", tag="g64")
            nc.tensor.matmul(
                g64_ps[:], e8_sb[:], s3in[EW:KW, :], start=True, stop=True
            )
            nc.vector.tensor_mul(s3in[0:EW, :], h1b[:], g64_ps[:])

        ENGS = ("s", "v", "s", "v")

        def back_chunk(i, j, s3in, out_sb, store_chunk):
            lhsT = s3in[:, j * 128:(j + 1) * 128]
            s3ab = s3ap.tile([128, W3N], f32, name="s3ab", tag="s3")
            nc.tensor.matmul(
                s3ab[:, 0:512], lhsT, w3_sb[:, 0:512], start=True, stop=True
            )
            nc.tensor.matmul(
                s3ab[:, 512:W3N], lhsT, w3_sb[:, 512:W3N], start=True, stop=True
            )
            if j in FILL_CHUNK:
                filler(x_sbs[i], 1)
            rc = small.tile([128, 1], f32, name="rc", tag="rc")
            nc.vector.reciprocal(rc[:], s3ab[:, 0:1])
            if ENGS[j] == "s":
                nc.scalar.mul(out_sb[:, j * D:(j + 1) * D], s3ab[:, 2:W3N], rc[:])
            else:
                nc.vector.tensor_scalar_mul(
                    out_sb[:, j * D:(j + 1) * D], s3ab[:, 2:W3N], rc[:]
                )
            if store_chunk:
                nc.sync.dma_start(
                    out_v[i, :, j, :], out_sb[:, j * D:(j + 1) * D]
                )

        def back(i, store_chunks=False):
            s3in = s3ins.pop(i)
            out_sb = outp.tile([128, JC * D], f16, name="out_sb", tag="out")
            for j in range(JC):
                back_chunk(i, j, s3in, out_sb, store_chunk=store_chunks)
            if not store_chunks:
                nc.sync.dma_start(
                    out_v[i, :, :, :],
                    out_sb[:].rearrange("p (j d) -> p j d", j=JC),
                )
            filler(x_sbs[i], FILL_END)
            x_sbs.pop(i)

        # software-pipelined emission. PE order per step:
        #   ..., s1(i+1), s3(i) chunks, g64(i+1), s1(i+2), s3(i+1), ...
        # so the V/S gate ladder of tile i+1 hides under s3(i) + s1(i+2).
        load(0)
        load(1)
        mid1(0)
        mid2(0)
        for i in range(N_TILES):
            if i + 1 < N_TILES:
                mid1(i + 1)
            back(i, store_chunks=(i >= N_TILES - 2))
            if i + 1 < N_TILES:
                mid2(i + 1)
            if i + 2 < N_TILES:
                load(i + 2)

    nc.compile()
    return nc


def _pack_host_inputs(Wd, bd, Wm, bm, Wu, bu, Wg, bg):
    """Repack the tiny weights into the on-chip layouts (host-side, ~100KB)."""
    f = np.float32
    W1 = np.concatenate(
        [np.ascontiguousarray(Wd.transpose(1, 0, 2)).reshape(D, EW), Wg], axis=1
    ).astype(f)                                   # [768, 72]
    w1p = np.ascontiguousarray(
        W1.reshape(KC, 128, KW).transpose(1, 0, 2)
    ).reshape(128, KC * KW)                       # [128, 432]; chunk c at cols c*72

    e8 = np.kron(np.eye(E, dtype=f), np.ones((1, R), f))   # [8, 64]

    wmbd = np.zeros((EW, EW), f)
    for e in range(E):
        wmbd[e * R:(e + 1) * R, e * R:(e + 1) * R] = Wm[e]
    wu = Wu.reshape(EW, D).astype(f)

    # stage-3 with stage-2 folded in: rows 0:64 = Wm_bd @ Wu_cat;
    # rows 64:72 = bu + bm_e @ Wu_e; cols 0,1 = Z (softmax denominator).
    w3e = np.zeros((KW, W3N), f)
    w3e[EW:, 0] = 1.0
    w3e[EW:, 1] = 1.0
    w3e[:EW, 2:] = wmbd @ wu
    w3e[EW:, 2:] = bu + np.einsum("er,erd->ed", bm, Wu)

    wpack = np.zeros((128, NW), np.float16)
    c0 = 0
    wpack[:, c0:c0 + KC * KW] = w1p; c0 += KC * KW
    wpack[EW:KW, c0:c0 + EW] = e8; c0 += EW
    wpack[0:KW, c0:c0 + W3N] = w3e; c0 += W3N
    wpack[0:EW, c0] = bd.reshape(EW); c0 += 1
    wpack[0:E, c0] = bg.reshape(E); c0 += 1
    return {"wpack": wpack}


def _pack_x(x_core16):
    """[T_CORE, D] fp16 -> tile-major transposed layout [128, NT*KC*T]."""
    return np.ascontiguousarray(
        x_core16.reshape(N_TILES, TILE_T, KC, 128).transpose(3, 0, 2, 1)
    ).reshape(128, N_TILES * KC * TILE_T)


def _run(inputs, trace=False, **kw):
    from concourse import bass_utils

    if "nc" not in _CACHE:
        _CACHE["nc"] = _build_and_compile()
    nc = _CACHE["nc"]

    x16 = np.asarray(inputs["x"], dtype=np.float32).reshape(
        NCORES, T_CORE, D
    ).astype(np.float16)
    w = _pack_host_inputs(
        *(np.asarray(inputs[k], dtype=np.float32)
          for k in ["Wd", "bd", "Wm", "bm", "Wu", "bu", "Wg", "bg"])
    )
    in_maps = [{"xt": _pack_x(x16[i]), **w} for i in range(NCORES)]
    res = bass_utils.run_bass_kernel_spmd(
        nc, in_maps, core_ids=list(range(NCORES)), trace=trace, **kw
    )
    out = np.concatenate(
        [res.results[i]["out"] for i in range(NCORES)], axis=0
    ).astype(np.float32).reshape(B, S, D)
    return out, res


def kernel(**inputs) -> np.ndarray:
    out, _ = _run(inputs)
    return out
